# revision 1
# baseline (speedup 1.0000x reference)
"""Grouped single-step GRU (16 independent GRU cells), Trainium2 Bass kernel.

Problem shapes (hardcoded): B=8192, U=16, I=H=128, fp32.
  gx = einsum("bui,ugi->bug", x, w_ih) + b_ih
  gh = einsum("buh,ugh->bug", h, w_hh) + b_hh
  r = sig(gx_r + gh_r); z = sig(gx_z + gh_z); n = tanh(gx_n + r * gh_n)
  out = (1-z)*n + z*h

Sharding: expert/unit-parallel — each of the 8 cores owns 2 units and the
full batch. This avoids replicating weights (only 0.75 MB of weights per
core) so per-core HBM traffic is ~24.75 MB, the problem's memory floor.

On-chip layout: gate dim (128) on partitions, batch on the free dim.
Host pre-transposes x/h to [u, feat, batch] so the contraction dim (feat)
lands on partitions for the TensorE moving operand; weights are
pre-transposed to [u, feat, 3H] to serve as stationary operands.
r/z gates accumulate their x- and h- matmuls in PSUM (2 matmuls, one bank);
the n gate keeps xn/hn separate (r gates hn before the sum).
"""

import os
import sys

import numpy as np

B, U, I, H = 8192, 16, 128, 128
N_CORES = 8
U_LOC = U // N_CORES  # units per core
BT = 512              # batch tile (free dim; one PSUM bank in fp32)
NT = B // BT

_CACHE = {}


def _import_concourse():
    try:
        import concourse.bass  # noqa: F401
    except ImportError:
        for p in ("/opt/trn_rl_repo", "/root/.axon_site/_ro/trn_rl_repo"):
            if os.path.isdir(p) and p not in sys.path:
                sys.path.insert(0, p)
        import concourse.bass  # noqa: F401


def _build():
    if "nc" in _CACHE:
        return _CACHE["nc"]
    _import_concourse()
    from contextlib import ExitStack

    import concourse.bacc as bacc
    import concourse.bass as bass
    import concourse.tile as tile
    from concourse import mybir

    f32 = mybir.dt.float32
    f32r = mybir.dt.float32r
    AFT = mybir.ActivationFunctionType
    ALU = mybir.AluOpType

    nc = bacc.Bacc(None, target_bir_lowering=False)
    x_t = nc.declare_dram_parameter("x_t", [U_LOC, I, B], f32r, isOutput=False)
    h_t = nc.declare_dram_parameter("h_t", [U_LOC, H, B], f32r, isOutput=False)
    wih = nc.declare_dram_parameter("wih", [U_LOC, I, 3 * H], f32r, isOutput=False)
    whh = nc.declare_dram_parameter("whh", [U_LOC, H, 3 * H], f32r, isOutput=False)
    bia = nc.declare_dram_parameter("bia", [H, U_LOC, 4], f32, isOutput=False)
    out_t = nc.declare_dram_parameter("out_t", [U_LOC, H, B], f32, isOutput=True)

    with ExitStack() as ctx:
        tc = ctx.enter_context(tile.TileContext(nc))
        wpool = ctx.enter_context(tc.tile_pool(name="w", bufs=1))
        xpool = ctx.enter_context(tc.tile_pool(name="x", bufs=6))
        hpool = ctx.enter_context(tc.tile_pool(name="h", bufs=6))
        tmp = ctx.enter_context(tc.tile_pool(name="tmp", bufs=4))
        opool = ctx.enter_context(tc.tile_pool(name="o", bufs=6))
        psum = ctx.enter_context(tc.tile_pool(name="psum", bufs=2, space="PSUM"))

        w_ih_sb = wpool.tile([I, U_LOC, 3 * H], f32r)
        w_hh_sb = wpool.tile([H, U_LOC, 3 * H], f32r)
        bias_sb = wpool.tile([H, U_LOC, 4], f32)
        nc.sync.dma_start(out=w_ih_sb, in_=wih[:].rearrange("u i g -> i u g"))
        nc.sync.dma_start(out=w_hh_sb, in_=whh[:].rearrange("u i g -> i u g"))
        nc.sync.dma_start(out=bias_sb, in_=bia[:])

        for u in range(U_LOC):
            for t in range(NT):
                bs = slice(t * BT, (t + 1) * BT)
                x_sb = xpool.tile([I, BT], f32r, tag="x")
                h_sb = hpool.tile([H, BT], f32r, tag="h")
                nc.sync.dma_start(out=x_sb, in_=x_t[u, :, bs])
                nc.sync.dma_start(out=h_sb, in_=h_t[u, :, bs])

                p_r = psum.tile([H, BT], f32, tag="pr")
                p_z = psum.tile([H, BT], f32, tag="pz")
                p_xn = psum.tile([H, BT], f32, tag="pxn")
                p_hn = psum.tile([H, BT], f32, tag="phn")

                # r/z: accumulate x- and h-contributions in PSUM.
                # fp32r: single-pass PE mode (4x faster than fp32's two
                # half-speed passes); accumulation stays fp32 in PSUM.
                xr_, hr_ = x_sb[:], h_sb[:]
                wi_, wh_ = w_ih_sb[:, u, :], w_hh_sb[:, u, :]
                nc.tensor.matmul(p_r, wi_[:, 0:H], xr_, start=True, stop=False)
                nc.tensor.matmul(p_r, wh_[:, 0:H], hr_, start=False, stop=True)
                nc.tensor.matmul(p_z, wi_[:, H:2 * H], xr_, start=True, stop=False)
                nc.tensor.matmul(p_z, wh_[:, H:2 * H], hr_, start=False, stop=True)
                nc.tensor.matmul(p_xn, wi_[:, 2 * H:], xr_, start=True, stop=True)
                nc.tensor.matmul(p_hn, wh_[:, 2 * H:], hr_, start=True, stop=True)

                r_sb = tmp.tile([H, BT], f32, tag="r")
                z_sb = tmp.tile([H, BT], f32, tag="z")
                nc.scalar.activation(out=r_sb, in_=p_r, func=AFT.Sigmoid,
                                     bias=bias_sb[:, u, 0:1])
                nc.scalar.activation(out=z_sb, in_=p_z, func=AFT.Sigmoid,
                                     bias=bias_sb[:, u, 1:2])
                # m = (hn + b_hhn) * r
                m_sb = tmp.tile([H, BT], f32, tag="m")
                nc.vector.scalar_tensor_tensor(
                    out=m_sb, in0=p_hn, scalar=bias_sb[:, u, 3:4], in1=r_sb,
                    op0=ALU.add, op1=ALU.mult)
                s_sb = tmp.tile([H, BT], f32, tag="s")
                nc.vector.tensor_add(s_sb, m_sb, p_xn)
                n_sb = tmp.tile([H, BT], f32, tag="n")
                nc.scalar.activation(out=n_sb, in_=s_sb, func=AFT.Tanh,
                                     bias=bias_sb[:, u, 2:3])
                # out = n + z*(h - n)
                d_sb = tmp.tile([H, BT], f32, tag="d")
                nc.gpsimd.tensor_sub(d_sb, h_sb[:].bitcast(f32), n_sb)
                zd_sb = tmp.tile([H, BT], f32, tag="zd")
                nc.vector.tensor_mul(zd_sb, z_sb, d_sb)
                o_sb = opool.tile([H, BT], f32, tag="o")
                nc.gpsimd.tensor_add(o_sb, n_sb, zd_sb)
                nc.sync.dma_start(out=out_t[u, :, bs], in_=o_sb)

    nc.compile()
    _CACHE["nc"] = nc
    return nc


def _make_in_maps(inputs, hidden, w_ih, w_hh, b_ih, b_hh):
    x_all = np.ascontiguousarray(inputs.transpose(1, 2, 0), dtype=np.float32)
    h_all = np.ascontiguousarray(hidden.transpose(1, 2, 0), dtype=np.float32)
    wihT = np.ascontiguousarray(w_ih.transpose(0, 2, 1), dtype=np.float32)
    whhT = np.ascontiguousarray(w_hh.transpose(0, 2, 1), dtype=np.float32)
    bias_r = (b_ih[:, :H] + b_hh[:, :H]).astype(np.float32)
    bias_z = (b_ih[:, H:2 * H] + b_hh[:, H:2 * H]).astype(np.float32)
    b_ihn = b_ih[:, 2 * H:].astype(np.float32)
    b_hhn = b_hh[:, 2 * H:].astype(np.float32)
    in_maps = []
    for c in range(N_CORES):
        us = slice(c * U_LOC, (c + 1) * U_LOC)
        bp = np.stack([bias_r[us], bias_z[us], b_ihn[us], b_hhn[us]], axis=-1)
        in_maps.append({
            "x_t": np.ascontiguousarray(x_all[us]),
            "h_t": np.ascontiguousarray(h_all[us]),
            "wih": np.ascontiguousarray(wihT[us]),
            "whh": np.ascontiguousarray(whhT[us]),
            "bia": np.ascontiguousarray(bp.transpose(1, 0, 2)),
        })
    return in_maps


def _run(in_maps, trace=False, **kw):
    _import_concourse()
    from concourse.bass_utils import run_bass_kernel_spmd

    nc = _build()
    return run_bass_kernel_spmd(nc, in_maps, list(range(N_CORES)), trace=trace, **kw)


def _assemble(res):
    out = np.concatenate([r["out_t"] for r in res.results], axis=0)  # (U, H, B)
    return np.ascontiguousarray(out.transpose(2, 0, 1))  # (B, U, H)


def kernel(**inputs):
    in_maps = _make_in_maps(
        np.asarray(inputs["inputs"]), np.asarray(inputs["hidden"]),
        np.asarray(inputs["w_ih"]), np.asarray(inputs["w_hh"]),
        np.asarray(inputs["b_ih"]), np.asarray(inputs["b_hh"]))
    return _assemble(_run(in_maps, trace=False))


def kernel_traced(inputs, **kw):
    """Test-harness entry: returns (output, BassKernelResults)."""
    in_maps = _make_in_maps(
        np.asarray(inputs["inputs"]), np.asarray(inputs["hidden"]),
        np.asarray(inputs["w_ih"]), np.asarray(inputs["w_hh"]),
        np.asarray(inputs["b_ih"]), np.asarray(inputs["b_hh"]))
    res = _run(in_maps, trace=True, **kw)
    return _assemble(res), res



# revision 2
# speedup vs baseline: 1.1480x; 1.1480x over previous
"""Grouped single-step GRU (16 independent GRU cells), Trainium2 Bass kernel, v13.

Shapes (hardcoded): B=8192, U=16, I=H=128; fp32 at the kernel() boundary,
fp16 device IO, fp32 PSUM/biases.

  r = sig(gx_r + gh_r); z = sig(gx_z + gh_z)
  n = tanh(gx_n + b_in + r * (gh_n + b_hn)); out = n + z*(h - n)

Sharding: expert/unit-parallel - each of 8 cores owns 2 units, full batch.

Key structure (per 1024-wide PSUM pair, 16 pairs/core):
 - PE: 12 gate matmuls + 2 identity matmuls that accumulate I @ m into the
   xn PSUM bank (start=False).  This replaces the DVE "s = xn + m" pass —
   the n-gate pre-activation forms entirely in PSUM, and tanh reads it
   with the b_in bias.  The identity matmuls are skewed one pair behind
   (m must exist first) so the PE never waits on the DVE.
 - Act: sig_r, sig_z, tanh — all 1024 wide.
 - DVE: m = (hn + b_hn) * r (stt, PSUM), zd = z*(h-n), o = n + zd (fp16).
 - Pool (gpsimd): d = h - n (fp16 tensor_tensor; gpsimd cannot read PSUM
   and has no scalar_tensor_tensor).
"""

import os
import sys

import numpy as np

B, U, I, H = 8192, 16, 128, 128
N_CORES = 8
U_LOC = U // N_CORES   # units per core
PT = 1024              # psum pair width (2 banks); also DMA granularity
NP = B // PT           # pairs per unit
_CACHE = {}


def _import_concourse():
    try:
        import concourse.bass  # noqa: F401
    except ImportError:
        for p in ("/opt/trn_rl_repo", "/root/.axon_site/_ro/trn_rl_repo"):
            if os.path.isdir(p) and p not in sys.path:
                sys.path.insert(0, p)
        import concourse.bass  # noqa: F401


def _build():
    if "nc" in _CACHE:
        return _CACHE["nc"]
    _import_concourse()
    from contextlib import ExitStack

    import concourse.bacc as bacc
    import concourse.tile as tile
    from concourse import mybir

    f32 = mybir.dt.float32
    f16 = mybir.dt.float16
    AFT = mybir.ActivationFunctionType
    ALU = mybir.AluOpType

    nc = bacc.Bacc(None, target_bir_lowering=False)
    x_t = nc.declare_dram_parameter("x_t", [U_LOC, I, B], f16, isOutput=False)
    h_t = nc.declare_dram_parameter("h_t", [U_LOC, H, B], f16, isOutput=False)
    wih = nc.declare_dram_parameter("wih", [U_LOC, I, 3 * H], f16, isOutput=False)
    whh = nc.declare_dram_parameter("whh", [U_LOC, H, 3 * H], f16, isOutput=False)
    bia = nc.declare_dram_parameter("bia", [H, U_LOC, 4], f32, isOutput=False)
    eye = nc.declare_dram_parameter("eye", [H, H], f16, isOutput=False)
    out_t = nc.declare_dram_parameter("out_t", [U_LOC, H, B], f16, isOutput=True)

    with ExitStack() as ctx:
        tc = ctx.enter_context(tile.TileContext(nc))
        wpool = ctx.enter_context(tc.tile_pool(name="w", bufs=1))
        xhpool = ctx.enter_context(tc.tile_pool(name="xh", bufs=4))
        gpool = ctx.enter_context(tc.tile_pool(name="g", bufs=3))
        opool = ctx.enter_context(tc.tile_pool(name="o", bufs=3))
        psum = ctx.enter_context(tc.tile_pool(name="psum", bufs=1, space="PSUM"))

        w_ih_sb = wpool.tile([I, U_LOC, 3 * H], f16)
        w_hh_sb = wpool.tile([H, U_LOC, 3 * H], f16)
        bias_sb = wpool.tile([H, U_LOC, 4], f32)
        eye_sb = wpool.tile([H, H], f16)
        # First pair's x/h jump the queue between the weight DMAs so the PE
        # can start as early as possible.
        nc.sync.dma_start(out=w_ih_sb, in_=wih[:].rearrange("u i g -> i u g"))
        x0_sb = xhpool.tile([I, PT], f16, tag="x")
        h0_sb = xhpool.tile([H, PT], f16, tag="h")
        nc.sync.dma_start(out=x0_sb, in_=x_t[0, :, 0:PT])
        nc.sync.dma_start(out=h0_sb, in_=h_t[0, :, 0:PT])
        nc.sync.dma_start(out=w_hh_sb, in_=whh[:].rearrange("u i g -> i u g"))
        nc.sync.dma_start(out=bias_sb, in_=bia[:])
        nc.sync.dma_start(out=eye_sb, in_=eye[:])

        # One pair of software-pipeline state: the I@m accumulation and
        # everything downstream of it run one pair behind the gate matmuls.
        pend = None  # (u, ps, h_sb, r..., see below)

        def flush(st):
            """Close the skewed pair: accumulate I@m into its xn bank, tanh,
            blend, and store."""
            u, ps, h_sb, z_p, m_p, p_xn = st
            b_in = bias_sb[:, u, 2:3]
            for t in range(2):
                nc.tensor.matmul(p_xn[:, t * 512:(t + 1) * 512], eye_sb[:],
                                 m_p[:, t * 512:(t + 1) * 512],
                                 start=False, stop=True, skip_group_check=True)
            n_p = gpool.tile([H, PT], f16, tag="n")
            nc.scalar.activation(out=n_p, in_=p_xn, func=AFT.Tanh, bias=b_in)
            # out = n + z*(h - n)
            d_p = gpool.tile([H, PT], f16, tag="d")
            nc.vector.tensor_sub(d_p, h_sb, n_p)
            zd_p = gpool.tile([H, PT], f16, tag="zd")
            nc.vector.tensor_mul(zd_p, z_p, d_p)
            o_p = opool.tile([H, PT], f16, tag="o")
            nc.vector.tensor_add(o_p, n_p, zd_p)
            nc.sync.dma_start(out=out_t[u, :, ps], in_=o_p)

        for u in range(U_LOC):
            wi, wh = w_ih_sb[:, u, :], w_hh_sb[:, u, :]
            b_r, b_z = bias_sb[:, u, 0:1], bias_sb[:, u, 1:2]
            b_hn = bias_sb[:, u, 3:4]
            for p in range(NP):
                ps = slice(p * PT, (p + 1) * PT)
                if u == 0 and p == 0:
                    x_sb, h_sb = x0_sb, h0_sb
                else:
                    x_sb = xhpool.tile([I, PT], f16, tag="x")
                    h_sb = xhpool.tile([H, PT], f16, tag="h")
                    nc.sync.dma_start(out=x_sb, in_=x_t[u, :, ps])
                    nc.sync.dma_start(out=h_sb, in_=h_t[u, :, ps])

                p_r = psum.tile([H, PT], f32, tag="pr")
                p_z = psum.tile([H, PT], f32, tag="pz")
                p_xn = psum.tile([H, PT], f32, tag="pxn")
                p_hn = psum.tile([H, PT], f32, tag="phn")
                xs = [x_sb[:, t * 512:(t + 1) * 512] for t in range(2)]
                hs = [h_sb[:, t * 512:(t + 1) * 512] for t in range(2)]
                # Same-stationary matmuls back-to-back (amortize weight
                # loads); r first (its sigmoid gates hn).
                for t in range(2):
                    nc.tensor.matmul(p_r[:, t * 512:(t + 1) * 512],
                                     wi[:, 0:H], xs[t], start=True, stop=False)
                for t in range(2):
                    nc.tensor.matmul(p_r[:, t * 512:(t + 1) * 512],
                                     wh[:, 0:H], hs[t], start=False, stop=True)
                for t in range(2):
                    nc.tensor.matmul(p_hn[:, t * 512:(t + 1) * 512],
                                     wh[:, 2 * H:], hs[t], start=True, stop=True)
                for t in range(2):
                    nc.tensor.matmul(p_z[:, t * 512:(t + 1) * 512],
                                     wi[:, H:2 * H], xs[t], start=True, stop=False)
                for t in range(2):
                    nc.tensor.matmul(p_z[:, t * 512:(t + 1) * 512],
                                     wh[:, H:2 * H], hs[t], start=False, stop=True)
                for t in range(2):
                    nc.tensor.matmul(p_xn[:, t * 512:(t + 1) * 512],
                                     wi[:, 2 * H:], xs[t], start=True, stop=False,
                                     skip_group_check=True)

                r_p = gpool.tile([H, PT], f16, tag="r")
                nc.scalar.activation(out=r_p, in_=p_r, func=AFT.Sigmoid,
                                     bias=b_r)
                # m = (hn + b_hn) * r
                m_p = gpool.tile([H, PT], f16, tag="m")
                nc.vector.scalar_tensor_tensor(
                    out=m_p, in0=p_hn, scalar=b_hn, in1=r_p,
                    op0=ALU.add, op1=ALU.mult)
                z_p = gpool.tile([H, PT], f16, tag="z")
                nc.scalar.activation(out=z_p, in_=p_z, func=AFT.Sigmoid,
                                     bias=b_z)

                if pend is not None:
                    flush(pend)
                pend = (u, ps, h_sb, z_p, m_p, p_xn)
        flush(pend)

    nc.compile()
    _CACHE["nc"] = nc
    return nc


def _make_in_maps(inputs, hidden, w_ih, w_hh, b_ih, b_hh):
    x_all = np.ascontiguousarray(inputs.transpose(1, 2, 0)).astype(np.float16)
    h_all = np.ascontiguousarray(hidden.transpose(1, 2, 0)).astype(np.float16)
    wihT = np.ascontiguousarray(w_ih.transpose(0, 2, 1)).astype(np.float16)
    whhT = np.ascontiguousarray(w_hh.transpose(0, 2, 1)).astype(np.float16)
    bias_r = (b_ih[:, :H] + b_hh[:, :H]).astype(np.float32)
    bias_z = (b_ih[:, H:2 * H] + b_hh[:, H:2 * H]).astype(np.float32)
    b_ihn = b_ih[:, 2 * H:].astype(np.float32)
    b_hhn = b_hh[:, 2 * H:].astype(np.float32)
    eye = np.eye(H, dtype=np.float16)
    in_maps = []
    for c in range(N_CORES):
        us = slice(c * U_LOC, (c + 1) * U_LOC)
        bp = np.stack([bias_r[us], bias_z[us], b_ihn[us], b_hhn[us]], axis=-1)
        in_maps.append({
            "x_t": np.ascontiguousarray(x_all[us]),
            "h_t": np.ascontiguousarray(h_all[us]),
            "wih": np.ascontiguousarray(wihT[us]),
            "whh": np.ascontiguousarray(whhT[us]),
            "bia": np.ascontiguousarray(bp.transpose(1, 0, 2)),
            "eye": eye,
        })
    return in_maps


def _run(in_maps, trace=False, **kw):
    _import_concourse()
    from concourse.bass_utils import run_bass_kernel_spmd

    nc = _build()
    return run_bass_kernel_spmd(nc, in_maps, list(range(N_CORES)), trace=trace, **kw)


def _assemble(res):
    out = np.concatenate([r["out_t"] for r in res.results], axis=0)  # (U, H, B) f16
    return np.ascontiguousarray(out.transpose(2, 0, 1)).astype(np.float32)


def kernel(**inputs):
    in_maps = _make_in_maps(
        np.asarray(inputs["inputs"]), np.asarray(inputs["hidden"]),
        np.asarray(inputs["w_ih"]), np.asarray(inputs["w_hh"]),
        np.asarray(inputs["b_ih"]), np.asarray(inputs["b_hh"]))
    return _assemble(_run(in_maps, trace=False))


def kernel_traced(inputs, **kw):
    """Test-harness entry: returns (output, BassKernelResults)."""
    in_maps = _make_in_maps(
        np.asarray(inputs["inputs"]), np.asarray(inputs["hidden"]),
        np.asarray(inputs["w_ih"]), np.asarray(inputs["w_hh"]),
        np.asarray(inputs["b_ih"]), np.asarray(inputs["b_hh"]))
    res = _run(in_maps, trace=True, **kw)
    return _assemble(res), res


# revision 3
# speedup vs baseline: 1.1662x; 1.0158x over previous
"""Grouped single-step GRU (16 independent GRU cells), Trainium2 Bass kernel, v21.

Shapes (hardcoded): B=8192, U=16, I=H=128; fp32 at the kernel() boundary,
fp16 device IO, fp32 PSUM/biases.

  r = sig(gx_r + gh_r); z = sig(gx_z + gh_z)
  n = tanh(gx_n + b_in + r * (gh_n + b_hn)); out = n + z*(h - n)

Sharding: expert/unit-parallel - each of 8 cores owns 2 units, full batch.

Key structure (per 1024-wide PSUM pair, 16 pairs/core):
 - PE: 12 gate matmuls + 2 identity matmuls that accumulate I @ m into the
   xn PSUM bank (start=False).  This replaces the DVE "s = xn + m" pass —
   the n-gate pre-activation forms entirely in PSUM, and tanh reads it
   with the b_in bias.  The identity matmuls are skewed one pair behind
   (m must exist first) so the PE never waits on the DVE.
 - Act: sig_r, sig_z, tanh — all 1024 wide.
 - DVE: m = (hn + b_hn) * r (stt, PSUM), zd = z*(h-n), o = n + zd (fp16).
 - Pool (gpsimd): d = h - n (fp16 tensor_tensor; gpsimd cannot read PSUM
   and has no scalar_tensor_tensor).
"""

import os
import sys

import numpy as np

B, U, I, H = 8192, 16, 128, 128
N_CORES = 8
U_LOC = U // N_CORES   # units per core
PT = 1024              # psum pair width (2 banks); also DMA granularity
NP = B // PT           # pairs per unit
_CACHE = {}


def _import_concourse():
    try:
        import concourse.bass  # noqa: F401
    except ImportError:
        for p in ("/opt/trn_rl_repo", "/root/.axon_site/_ro/trn_rl_repo"):
            if os.path.isdir(p) and p not in sys.path:
                sys.path.insert(0, p)
        import concourse.bass  # noqa: F401


def _build():
    if "nc" in _CACHE:
        return _CACHE["nc"]
    _import_concourse()
    from contextlib import ExitStack

    import concourse.bacc as bacc
    import concourse.tile as tile
    from concourse import mybir

    f32 = mybir.dt.float32
    f16 = mybir.dt.float16
    AFT = mybir.ActivationFunctionType
    ALU = mybir.AluOpType

    nc = bacc.Bacc(None, target_bir_lowering=False)
    x_t = nc.declare_dram_parameter("x_t", [U_LOC, I, B], f16, isOutput=False)
    h_t = nc.declare_dram_parameter("h_t", [U_LOC, H, B], f16, isOutput=False)
    wih = nc.declare_dram_parameter("wih", [U_LOC, I, 3 * H], f16, isOutput=False)
    whh = nc.declare_dram_parameter("whh", [U_LOC, H, 3 * H], f16, isOutput=False)
    bia = nc.declare_dram_parameter("bia", [H, U_LOC, 4], f32, isOutput=False)
    eye = nc.declare_dram_parameter("eye", [H, H], f16, isOutput=False)
    out_t = nc.declare_dram_parameter("out_t", [U_LOC, H, B], f16, isOutput=True)

    with ExitStack() as ctx:
        tc = ctx.enter_context(tile.TileContext(nc))
        wpool = ctx.enter_context(tc.tile_pool(name="w", bufs=1))
        xhpool = ctx.enter_context(tc.tile_pool(name="xh", bufs=4))
        gpool = ctx.enter_context(tc.tile_pool(name="g", bufs=3))
        opool = ctx.enter_context(tc.tile_pool(name="o", bufs=3))
        psum = ctx.enter_context(tc.tile_pool(name="psum", bufs=1, space="PSUM"))

        w_ih_sb = wpool.tile([I, U_LOC, 3 * H], f16)
        w_hh_sb = wpool.tile([H, U_LOC, 3 * H], f16)
        bias_sb = wpool.tile([H, U_LOC, 4], f32)
        eye_sb = wpool.tile([H, H], f16)
        # First pair's x/h jump the queue between the weight DMAs so the PE
        # can start as early as possible.
        nc.sync.dma_start(out=w_ih_sb, in_=wih[:].rearrange("u i g -> i u g"))
        x0_sb = xhpool.tile([I, PT], f16, tag="x")
        h0_sb = xhpool.tile([H, PT], f16, tag="h")
        nc.sync.dma_start(out=x0_sb, in_=x_t[0, :, 0:PT])
        nc.sync.dma_start(out=h0_sb, in_=h_t[0, :, 0:PT])
        nc.sync.dma_start(out=w_hh_sb, in_=whh[:].rearrange("u i g -> i u g"))
        # Pair 1's inputs jump ahead of the biases (first needed by the
        # pair-0 sigmoid, much later) and the identity (first needed by the
        # pair-0 flush, issued during pair 1) to cut the serial SP issue
        # chain ahead of pair 1's matmuls.
        x1_sb = xhpool.tile([I, PT], f16, tag="x")
        h1_sb = xhpool.tile([H, PT], f16, tag="h")
        nc.sync.dma_start(out=x1_sb, in_=x_t[0, :, PT:2 * PT])
        nc.sync.dma_start(out=h1_sb, in_=h_t[0, :, PT:2 * PT])
        nc.sync.dma_start(out=bias_sb, in_=bia[:])
        nc.sync.dma_start(out=eye_sb, in_=eye[:])

        # One pair of software-pipeline state: the I@m accumulation and
        # everything downstream of it run one pair behind the gate matmuls.
        pend = None  # (u, ps, h_sb, r..., see below)

        def flush(st, last=False):
            """Close the skewed pair: accumulate I@m into its xn bank, tanh,
            blend, and store.  The final pair runs its tail as two 512-wide
            halves so the serial drain chain at kernel end is shorter."""
            u, ps, h_sb, z_p, m_p, p_xn = st
            b_in = bias_sb[:, u, 2:3]
            for t in range(2):
                nc.tensor.matmul(p_xn[:, t * 512:(t + 1) * 512], eye_sb[:],
                                 m_p[:, t * 512:(t + 1) * 512],
                                 start=False, stop=True, skip_group_check=True)
            n_p = gpool.tile([H, PT], f16, tag="n")
            d_p = gpool.tile([H, PT], f16, tag="d")
            zd_p = gpool.tile([H, PT], f16, tag="zd")
            o_p = opool.tile([H, PT], f16, tag="o")
            for w0, w1 in ([(0, PT)] if not last else [(0, 512), (512, PT)]):
                sl = slice(w0, w1)
                nc.scalar.activation(out=n_p[:, sl], in_=p_xn[:, sl],
                                     func=AFT.Tanh, bias=b_in)
                # out = n + z*(h - n)
                nc.vector.tensor_sub(d_p[:, sl], h_sb[:, sl], n_p[:, sl])
                nc.vector.tensor_mul(zd_p[:, sl], z_p[:, sl], d_p[:, sl])
                nc.vector.tensor_add(o_p[:, sl], n_p[:, sl], zd_p[:, sl])
                nc.sync.dma_start(out=out_t[u, :, ps.start + w0:ps.start + w1],
                                  in_=o_p[:, sl])

        for u in range(U_LOC):
            wi, wh = w_ih_sb[:, u, :], w_hh_sb[:, u, :]
            b_r, b_z = bias_sb[:, u, 0:1], bias_sb[:, u, 1:2]
            b_hn = bias_sb[:, u, 3:4]
            for p in range(NP):
                ps = slice(p * PT, (p + 1) * PT)
                if u == 0 and p == 0:
                    x_sb, h_sb = x0_sb, h0_sb
                elif u == 0 and p == 1:
                    x_sb, h_sb = x1_sb, h1_sb
                else:
                    x_sb = xhpool.tile([I, PT], f16, tag="x")
                    h_sb = xhpool.tile([H, PT], f16, tag="h")
                    nc.sync.dma_start(out=x_sb, in_=x_t[u, :, ps])
                    nc.sync.dma_start(out=h_sb, in_=h_t[u, :, ps])

                p_r = psum.tile([H, PT], f32, tag="pr")
                p_z = psum.tile([H, PT], f32, tag="pz")
                p_xn = psum.tile([H, PT], f32, tag="pxn")
                p_hn = psum.tile([H, PT], f32, tag="phn")
                xs = [x_sb[:, t * 512:(t + 1) * 512] for t in range(2)]
                hs = [h_sb[:, t * 512:(t + 1) * 512] for t in range(2)]
                # Same-stationary matmuls back-to-back (amortize weight
                # loads); r first (its sigmoid gates hn).
                for t in range(2):
                    nc.tensor.matmul(p_r[:, t * 512:(t + 1) * 512],
                                     wi[:, 0:H], xs[t], start=True, stop=False)
                for t in range(2):
                    nc.tensor.matmul(p_r[:, t * 512:(t + 1) * 512],
                                     wh[:, 0:H], hs[t], start=False, stop=True)
                for t in range(2):
                    nc.tensor.matmul(p_hn[:, t * 512:(t + 1) * 512],
                                     wh[:, 2 * H:], hs[t], start=True, stop=True)
                for t in range(2):
                    nc.tensor.matmul(p_z[:, t * 512:(t + 1) * 512],
                                     wi[:, H:2 * H], xs[t], start=True, stop=False)
                for t in range(2):
                    nc.tensor.matmul(p_z[:, t * 512:(t + 1) * 512],
                                     wh[:, H:2 * H], hs[t], start=False, stop=True)
                for t in range(2):
                    nc.tensor.matmul(p_xn[:, t * 512:(t + 1) * 512],
                                     wi[:, 2 * H:], xs[t], start=True, stop=False,
                                     skip_group_check=True)

                r_p = gpool.tile([H, PT], f16, tag="r")
                nc.scalar.activation(out=r_p, in_=p_r, func=AFT.Sigmoid,
                                     bias=b_r)
                # m = (hn + b_hn) * r
                m_p = gpool.tile([H, PT], f16, tag="m")
                nc.vector.scalar_tensor_tensor(
                    out=m_p, in0=p_hn, scalar=b_hn, in1=r_p,
                    op0=ALU.add, op1=ALU.mult)
                z_p = gpool.tile([H, PT], f16, tag="z")
                nc.scalar.activation(out=z_p, in_=p_z, func=AFT.Sigmoid,
                                     bias=b_z)

                if pend is not None:
                    flush(pend)
                pend = (u, ps, h_sb, z_p, m_p, p_xn)
        flush(pend, last=True)

    nc.compile()
    _CACHE["nc"] = nc
    return nc


def _make_in_maps(inputs, hidden, w_ih, w_hh, b_ih, b_hh):
    x_all = np.ascontiguousarray(inputs.transpose(1, 2, 0)).astype(np.float16)
    h_all = np.ascontiguousarray(hidden.transpose(1, 2, 0)).astype(np.float16)
    wihT = np.ascontiguousarray(w_ih.transpose(0, 2, 1)).astype(np.float16)
    whhT = np.ascontiguousarray(w_hh.transpose(0, 2, 1)).astype(np.float16)
    bias_r = (b_ih[:, :H] + b_hh[:, :H]).astype(np.float32)
    bias_z = (b_ih[:, H:2 * H] + b_hh[:, H:2 * H]).astype(np.float32)
    b_ihn = b_ih[:, 2 * H:].astype(np.float32)
    b_hhn = b_hh[:, 2 * H:].astype(np.float32)
    eye = np.eye(H, dtype=np.float16)
    in_maps = []
    for c in range(N_CORES):
        us = slice(c * U_LOC, (c + 1) * U_LOC)
        bp = np.stack([bias_r[us], bias_z[us], b_ihn[us], b_hhn[us]], axis=-1)
        in_maps.append({
            "x_t": np.ascontiguousarray(x_all[us]),
            "h_t": np.ascontiguousarray(h_all[us]),
            "wih": np.ascontiguousarray(wihT[us]),
            "whh": np.ascontiguousarray(whhT[us]),
            "bia": np.ascontiguousarray(bp.transpose(1, 0, 2)),
            "eye": eye,
        })
    return in_maps


def _run(in_maps, trace=False, **kw):
    _import_concourse()
    from concourse.bass_utils import run_bass_kernel_spmd

    nc = _build()
    return run_bass_kernel_spmd(nc, in_maps, list(range(N_CORES)), trace=trace, **kw)


def _assemble(res):
    out = np.concatenate([r["out_t"] for r in res.results], axis=0)  # (U, H, B) f16
    return np.ascontiguousarray(out.transpose(2, 0, 1)).astype(np.float32)


def kernel(**inputs):
    in_maps = _make_in_maps(
        np.asarray(inputs["inputs"]), np.asarray(inputs["hidden"]),
        np.asarray(inputs["w_ih"]), np.asarray(inputs["w_hh"]),
        np.asarray(inputs["b_ih"]), np.asarray(inputs["b_hh"]))
    return _assemble(_run(in_maps, trace=False))


def kernel_traced(inputs, **kw):
    """Test-harness entry: returns (output, BassKernelResults)."""
    in_maps = _make_in_maps(
        np.asarray(inputs["inputs"]), np.asarray(inputs["hidden"]),
        np.asarray(inputs["w_ih"]), np.asarray(inputs["w_hh"]),
        np.asarray(inputs["b_ih"]), np.asarray(inputs["b_hh"]))
    res = _run(in_maps, trace=True, **kw)
    return _assemble(res), res


# revision 4
# speedup vs baseline: 1.1889x; 1.0195x over previous
"""Grouped single-step GRU (16 independent GRU cells), Trainium2 Bass kernel, v21.

Shapes (hardcoded): B=8192, U=16, I=H=128; fp32 at the kernel() boundary,
fp16 device IO, fp32 PSUM/biases.

  r = sig(gx_r + gh_r); z = sig(gx_z + gh_z)
  n = tanh(gx_n + b_in + r * (gh_n + b_hn)); out = n + z*(h - n)

Sharding: expert/unit-parallel - each of 8 cores owns 2 units, full batch.

Key structure (per 1024-wide PSUM pair, 16 pairs/core):
 - PE: 12 gate matmuls + 2 identity matmuls that accumulate I @ m into the
   xn PSUM bank (start=False).  This replaces the DVE "s = xn + m" pass —
   the n-gate pre-activation forms entirely in PSUM, and tanh reads it
   with the b_in bias.  The identity matmuls are skewed one pair behind
   (m must exist first) so the PE never waits on the DVE.
 - Act: sig_r, sig_z, tanh — all 1024 wide.
 - DVE: m = (hn + b_hn) * r (stt, PSUM), zd = z*(h-n), o = n + zd (fp16).
 - Pool (gpsimd): d = h - n (fp16 tensor_tensor; gpsimd cannot read PSUM
   and has no scalar_tensor_tensor).
"""

import os
import sys

import numpy as np

B, U, I, H = 8192, 16, 128, 128
N_CORES = 8
U_LOC = U // N_CORES   # units per core
PT = 1024              # psum pair width (2 banks); also DMA granularity
NP = B // PT           # pairs per unit
_CACHE = {}


def _import_concourse():
    try:
        import concourse.bass  # noqa: F401
    except ImportError:
        for p in ("/opt/trn_rl_repo", "/root/.axon_site/_ro/trn_rl_repo"):
            if os.path.isdir(p) and p not in sys.path:
                sys.path.insert(0, p)
        import concourse.bass  # noqa: F401


def _build():
    if "nc" in _CACHE:
        return _CACHE["nc"]
    _import_concourse()
    from contextlib import ExitStack

    import concourse.bacc as bacc
    import concourse.tile as tile
    from concourse import mybir

    f32 = mybir.dt.float32
    f16 = mybir.dt.float16
    AFT = mybir.ActivationFunctionType
    ALU = mybir.AluOpType

    nc = bacc.Bacc(None, target_bir_lowering=False)
    x_t = nc.declare_dram_parameter("x_t", [U_LOC, I, B], f16, isOutput=False)
    h_t = nc.declare_dram_parameter("h_t", [U_LOC, H, B], f16, isOutput=False)
    wih = nc.declare_dram_parameter("wih", [U_LOC, I, 3 * H], f16, isOutput=False)
    whh = nc.declare_dram_parameter("whh", [U_LOC, H, 3 * H], f16, isOutput=False)
    bia = nc.declare_dram_parameter("bia", [H, U_LOC, 4], f32, isOutput=False)
    eye = nc.declare_dram_parameter("eye", [H, H], f16, isOutput=False)
    out_t = nc.declare_dram_parameter("out_t", [U_LOC, H, B], f16, isOutput=True)

    with ExitStack() as ctx:
        tc = ctx.enter_context(tile.TileContext(nc))
        wpool = ctx.enter_context(tc.tile_pool(name="w", bufs=1))
        xhpool = ctx.enter_context(tc.tile_pool(name="xh", bufs=4))
        gpool = ctx.enter_context(tc.tile_pool(name="g", bufs=3))
        opool = ctx.enter_context(tc.tile_pool(name="o", bufs=3))
        psum = ctx.enter_context(tc.tile_pool(name="psum", bufs=1, space="PSUM"))

        w_ih_sb = wpool.tile([I, U_LOC, 3 * H], f16)
        w_hh_sb = wpool.tile([H, U_LOC, 3 * H], f16)
        bias_sb = wpool.tile([H, U_LOC, 4], f32)
        eye_sb = wpool.tile([H, H], f16)
        # First pair's x/h jump the queue between the weight DMAs so the PE
        # can start as early as possible.
        nc.sync.dma_start(out=w_ih_sb, in_=wih[:].rearrange("u i g -> i u g"))
        x0_sb = xhpool.tile([I, PT], f16, tag="x")
        h0_sb = xhpool.tile([H, PT], f16, tag="h")
        nc.sync.dma_start(out=x0_sb, in_=x_t[0, :, 0:PT])
        nc.sync.dma_start(out=h0_sb, in_=h_t[0, :, 0:PT])
        nc.sync.dma_start(out=w_hh_sb, in_=whh[:].rearrange("u i g -> i u g"))
        # Pair 1's inputs jump ahead of the biases (first needed by the
        # pair-0 sigmoid, much later) and the identity (first needed by the
        # pair-0 flush, issued during pair 1) to cut the serial SP issue
        # chain ahead of pair 1's matmuls.
        x1_sb = xhpool.tile([I, PT], f16, tag="x")
        h1_sb = xhpool.tile([H, PT], f16, tag="h")
        nc.sync.dma_start(out=x1_sb, in_=x_t[0, :, PT:2 * PT])
        nc.sync.dma_start(out=h1_sb, in_=h_t[0, :, PT:2 * PT])
        nc.sync.dma_start(out=bias_sb, in_=bia[:])
        nc.sync.dma_start(out=eye_sb, in_=eye[:])

        # One pair of software-pipeline state: the I@m accumulation and
        # everything downstream of it run one pair behind the gate matmuls.
        pend = None  # (u, ps, h_sb, r..., see below)

        def flush(st, last=False):
            """Close the skewed pair: accumulate I@m into its xn bank, tanh,
            blend, and store.  The final pair runs its tail as two 512-wide
            halves so the serial drain chain at kernel end is shorter."""
            u, ps, h_sb, z_p, m_p, p_xn = st
            b_in = bias_sb[:, u, 2:3]
            for t in range(2):
                nc.tensor.matmul(p_xn[:, t * 512:(t + 1) * 512], eye_sb[:],
                                 m_p[:, t * 512:(t + 1) * 512],
                                 start=False, stop=True, skip_group_check=True)
            n_p = gpool.tile([H, PT], f16, tag="n")
            d_p = gpool.tile([H, PT], f16, tag="d")
            zd_p = gpool.tile([H, PT], f16, tag="zd")
            o_p = opool.tile([H, PT], f16, tag="o")
            for w0, w1 in ([(0, PT)] if not last else [(0, 512), (512, PT)]):
                sl = slice(w0, w1)
                nc.scalar.activation(out=n_p[:, sl], in_=p_xn[:, sl],
                                     func=AFT.Tanh, bias=b_in)
                # out = n + z*(h - n)
                nc.vector.tensor_sub(d_p[:, sl], h_sb[:, sl], n_p[:, sl])
                nc.vector.tensor_mul(zd_p[:, sl], z_p[:, sl], d_p[:, sl])
                nc.vector.tensor_add(o_p[:, sl], n_p[:, sl], zd_p[:, sl])
                nc.sync.dma_start(out=out_t[u, :, ps.start + w0:ps.start + w1],
                                  in_=o_p[:, sl])

        for u in range(U_LOC):
            wi, wh = w_ih_sb[:, u, :], w_hh_sb[:, u, :]
            b_r, b_z = bias_sb[:, u, 0:1], bias_sb[:, u, 1:2]
            b_hn = bias_sb[:, u, 3:4]
            for p in range(NP):
                ps = slice(p * PT, (p + 1) * PT)
                if u == 0 and p == 0:
                    x_sb, h_sb = x0_sb, h0_sb
                elif u == 0 and p == 1:
                    x_sb, h_sb = x1_sb, h1_sb
                else:
                    x_sb = xhpool.tile([I, PT], f16, tag="x")
                    h_sb = xhpool.tile([H, PT], f16, tag="h")
                    nc.sync.dma_start(out=x_sb, in_=x_t[u, :, ps])
                    nc.sync.dma_start(out=h_sb, in_=h_t[u, :, ps])

                p_r = psum.tile([H, PT], f32, tag="pr")
                p_z = psum.tile([H, PT], f32, tag="pz")
                p_xn = psum.tile([H, PT], f32, tag="pxn")
                p_hn = psum.tile([H, PT], f32, tag="phn")
                xs = [x_sb[:, t * 512:(t + 1) * 512] for t in range(2)]
                hs = [h_sb[:, t * 512:(t + 1) * 512] for t in range(2)]
                # Same-stationary matmuls back-to-back (amortize weight
                # loads); r first (its sigmoid gates hn).
                for t in range(2):
                    nc.tensor.matmul(p_r[:, t * 512:(t + 1) * 512],
                                     wi[:, 0:H], xs[t], start=True, stop=False)
                for t in range(2):
                    nc.tensor.matmul(p_r[:, t * 512:(t + 1) * 512],
                                     wh[:, 0:H], hs[t], start=False, stop=True)
                for t in range(2):
                    nc.tensor.matmul(p_hn[:, t * 512:(t + 1) * 512],
                                     wh[:, 2 * H:], hs[t], start=True, stop=True)
                for t in range(2):
                    nc.tensor.matmul(p_z[:, t * 512:(t + 1) * 512],
                                     wi[:, H:2 * H], xs[t], start=True, stop=False)
                for t in range(2):
                    nc.tensor.matmul(p_z[:, t * 512:(t + 1) * 512],
                                     wh[:, H:2 * H], hs[t], start=False, stop=True)
                for t in range(2):
                    nc.tensor.matmul(p_xn[:, t * 512:(t + 1) * 512],
                                     wi[:, 2 * H:], xs[t], start=True, stop=False,
                                     skip_group_check=True)

                r_p = gpool.tile([H, PT], f16, tag="r")
                nc.scalar.activation(out=r_p, in_=p_r, func=AFT.Sigmoid,
                                     bias=b_r)
                # m = (hn + b_hn) * r
                m_p = gpool.tile([H, PT], f16, tag="m")
                nc.vector.scalar_tensor_tensor(
                    out=m_p, in0=p_hn, scalar=b_hn, in1=r_p,
                    op0=ALU.add, op1=ALU.mult)
                z_p = gpool.tile([H, PT], f16, tag="z")
                nc.scalar.activation(out=z_p, in_=p_z, func=AFT.Sigmoid,
                                     bias=b_z)

                if pend is not None:
                    flush(pend)
                pend = (u, ps, h_sb, z_p, m_p, p_xn)
        flush(pend, last=True)

    nc.compile()
    _CACHE["nc"] = nc
    return nc


def _make_in_maps(inputs, hidden, w_ih, w_hh, b_ih, b_hh):
    x_all = np.ascontiguousarray(inputs.transpose(1, 2, 0)).astype(np.float16)
    h_all = np.ascontiguousarray(hidden.transpose(1, 2, 0)).astype(np.float16)
    wihT = np.ascontiguousarray(w_ih.transpose(0, 2, 1)).astype(np.float16)
    whhT = np.ascontiguousarray(w_hh.transpose(0, 2, 1)).astype(np.float16)
    bias_r = (b_ih[:, :H] + b_hh[:, :H]).astype(np.float32)
    bias_z = (b_ih[:, H:2 * H] + b_hh[:, H:2 * H]).astype(np.float32)
    b_ihn = b_ih[:, 2 * H:].astype(np.float32)
    b_hhn = b_hh[:, 2 * H:].astype(np.float32)
    eye = np.eye(H, dtype=np.float16)
    in_maps = []
    for c in range(N_CORES):
        us = slice(c * U_LOC, (c + 1) * U_LOC)
        bp = np.stack([bias_r[us], bias_z[us], b_ihn[us], b_hhn[us]], axis=-1)
        in_maps.append({
            "x_t": np.ascontiguousarray(x_all[us]),
            "h_t": np.ascontiguousarray(h_all[us]),
            "wih": np.ascontiguousarray(wihT[us]),
            "whh": np.ascontiguousarray(whhT[us]),
            "bia": np.ascontiguousarray(bp.transpose(1, 0, 2)),
            "eye": eye,
        })
    return in_maps


def _run(in_maps, trace=False, **kw):
    _import_concourse()
    from concourse.bass_utils import run_bass_kernel_spmd

    nc = _build()
    return run_bass_kernel_spmd(nc, in_maps, list(range(N_CORES)), trace=trace, **kw)


def _assemble(res):
    out = np.concatenate([r["out_t"] for r in res.results], axis=0)  # (U, H, B) f16
    return np.ascontiguousarray(out.transpose(2, 0, 1)).astype(np.float32)


def kernel(**inputs):
    in_maps = _make_in_maps(
        np.asarray(inputs["inputs"]), np.asarray(inputs["hidden"]),
        np.asarray(inputs["w_ih"]), np.asarray(inputs["w_hh"]),
        np.asarray(inputs["b_ih"]), np.asarray(inputs["b_hh"]))
    try:
        return _assemble(_run(in_maps, trace=False))
    except Exception:
        # The device occasionally reports a transient unrecoverable state on
        # the first touch after a previous process; one retry clears it.
        return _assemble(_run(in_maps, trace=False))


def kernel_traced(inputs, **kw):
    """Test-harness entry: returns (output, BassKernelResults)."""
    in_maps = _make_in_maps(
        np.asarray(inputs["inputs"]), np.asarray(inputs["hidden"]),
        np.asarray(inputs["w_ih"]), np.asarray(inputs["w_hh"]),
        np.asarray(inputs["b_ih"]), np.asarray(inputs["b_hh"]))
    res = _run(in_maps, trace=True, **kw)
    return _assemble(res), res
